# revision 18
# baseline (speedup 1.0000x reference)
"""Trainium2 Bass kernel: GRU encoder + beam-search decoder (V=32000,H=512,L=2,BW=5).

All 8 cores run an identical SPMD program. Encoder + decoder GRU/attention/Wc
replicated per core in fp16 weights/activations with fp32 accumulation
(host-verified to reproduce the reference token sequence). Output projection is
vocab-sharded (4000/core) in fp32; per-core per-beam top-8 candidates and
exp-sum partials are AllGathered every step and each core performs the
identical merge. Backtrack runs on host.
"""
import os
import sys
import numpy as np

sys.path.insert(0, "/opt/trn_rl_repo")

import concourse.bass as bass
import concourse.bacc as bacc
import concourse.mybir as mybir
from concourse.tile import TileContext
from concourse.bass_utils import run_bass_kernel_spmd

F32 = mybir.dt.float32
F16 = mybir.dt.float16
U32 = mybir.dt.uint32
U16 = mybir.dt.uint16
I32 = mybir.dt.int32
AF = mybir.ActivationFunctionType
ALU = mybir.AluOpType

V, H, L, BW = 32000, 512, 2, 5
NCORE = 8
VS = V // NCORE
NEG = -1e9

_CACHE = {}


def _tile_wT(W):
    """W: [G,K] fp32 (g = W @ x). -> [128, MC*KC*128] lhsT tile layout:
    tile (m,k) at cols (m*KC+k)*128 ; value[p, c] = W[128m+c, 128k+p]."""
    G, K = W.shape
    MC, KC = G // 128, K // 128
    out = np.zeros((128, MC * KC * 128), np.float32)
    for m in range(MC):
        for k in range(KC):
            blk = W[m * 128:(m + 1) * 128, k * 128:(k + 1) * 128]
            out[:, (m * KC + k) * 128:(m * KC + k + 1) * 128] = blk.T
    return out


def build(T, S):
    nc = bacc.Bacc(None, target_bir_lowering=False)
    din = {}

    def dram_in(name, shape, dt=F32):
        din[name] = nc.dram_tensor(name, shape, dt, kind="ExternalInput")
        return din[name]

    for nm in ["encWh0", "encWh1", "encWi0", "encWi1",
               "decWi0", "decWh0", "decWi1", "decWh1"]:
        dram_in(nm, [128, 12 * 4 * 128], F16)
    dram_in("WcT", [128, 4 * 8 * 128], F16)
    dram_in("WoutT", [128, 4 * VS], F32)
    dram_in("xT", [128, 4 * T], F16)
    dram_in("enc_gib0", [1, 12 * 128])
    dram_in("enc_gib1", [1, 12 * 128])
    dram_in("enc_bhn0", [128, 4])
    dram_in("enc_bhn1", [128, 4])
    for nm in ["dec_brz0", "dec_brz1"]:
        dram_in(nm, [1, 8 * 128])
    for nm in ["dec_bin0", "dec_bin1", "dec_bhn0", "dec_bhn1", "bcr"]:
        dram_in(nm, [1, 4 * 128])
    dram_in("boutRep", [128, 2 * 500])
    dram_in("ident", [128, 128])
    dram_in("ident16", [128, 128], F16)
    dram_in("ones15", [1, 5])
    dram_in("onesT", [1, T])
    dram_in("Icc", [128, 20])
    dram_in("Isum", [128, 5])
    dram_in("Ir", [40, 40])
    dram_in("IrSum", [40, 5])
    dram_in("Ib", [16, 5])
    dram_in("voffA", [128, 1])
    dram_in("voffB", [128, 1])
    dram_in("base40", [1, 40])
    dram_in("rowidx40", [64, 1])
    dram_in("ones40", [1, 40])
    dram_in("h0sc", [16, 1])
    dram_in("xd0", [128, 4 * 5], F16)
    dram_in("emb", [V, H], F32)

    out_tp = nc.dram_tensor("out_tp", [2, S * 5], F32, kind="ExternalOutput")
    gdram = nc.dram_tensor("gdram", [5 * 512, 1], F32)
    out_sc = nc.dram_tensor("out_sc", [5, 1], F32, kind="ExternalOutput")


    with TileContext(nc) as tc:
        with tc.tile_pool(name="w", bufs=1) as wp, \
             tc.tile_pool(name="wk", bufs=1) as wk, \
             tc.tile_pool(name="dramp", bufs=2, space="DRAM") as dramp, \
             tc.tile_pool(name="ps", bufs=1, space="PSUM") as ps:

            sb = {}
            DEFER = ("decWi0", "decWh0", "decWi1", "decWh1", "emb")
            WTAG = {"encWi0": "wbig0", "encWh0": "wbig1",
                    "encWi1": "wbig2", "encWh1": "wbig3"}
            for name, t in din.items():
                if name in DEFER:
                    continue
                sb[name] = wp.tile(list(t.shape), t.dtype,
                                   tag=WTAG.get(name, name), name=name)
                nc.sync.dma_start(out=sb[name][:], in_=t[:])

            def w4(name):
                return sb[name].rearrange("p (m k c) -> p m k c", k=4, c=128)

            ident = sb["ident"]

            hT = wp.tile([128, 2 * 4], F32, tag="hT")
            l0OT16 = wp.tile([128, 4 * T], F16, tag="l0OT16")
            encOT16 = wp.tile([128, 4 * T], F16, tag="encOT16")
            encNat = wp.tile([T, 4 * 128], F16, tag="encNat")
            gi = wp.tile([128, 12 * T], F32, tag="gi")
            hdT = wp.tile([128, 2 * 4 * 5], F32, tag="hdT")
            hdT16 = wp.tile([128, 2 * 4 * 5], F16, tag="hdT16")
            xdT = wp.tile([128, 4 * 5], F16, tag="xdT")
            scores = wp.tile([16, 1], F32, tag="scores")
            lse = wp.tile([16, 1], F32, tag="lse")
            toksAcc = wp.tile([16, S], F32, tag="toksAcc")
            parsAcc = wp.tile([16, S], F32, tag="parsAcc")
            cflat = wp.tile([1, 5 * 512], F32, tag="cflat")
            gflat = wp.tile([1, 5 * 512], F32, tag="gflat")
            mv = wp.tile([16, 1024], F32, tag="mv")
            gsb = wp.tile([40, 136], F32, tag="gsb")
            payload = wp.tile([5, 136], F32, tag="payload")
            idxt = wp.tile([16, 1], I32, tag="idxt")
            tt10 = wp.tile([1, 16], F32, tag="tt10")

            nc.vector.tensor_copy(scores[:], sb["h0sc"][:])
            nc.gpsimd.memset(hT[:], 0.0)
            nc.gpsimd.memset(mv[:], NEG)
            nc.gpsimd.memset(idxt[:], 0)
            nc.gpsimd.memset(lse[:], 0.0)
            nc.vector.tensor_copy(xdT[:], sb["xd0"][:])

            # =============== ENCODER ===============
            xTr = sb["xT"].rearrange("p (k t) -> p k t", k=4)
            for lay in range(2):
                wi = w4("encWi%d" % lay)
                gib = sb["enc_gib%d" % lay]
                for m in range(12):
                    pst = ps.tile([128, T], F32, tag="pA")
                    for k in range(4):
                        rhs = xTr[:, k, :] if lay == 0 else \
                            l0OT16.rearrange("p (k t) -> p k t", k=4)[:, k, :]
                        nc.tensor.matmul(pst[:], wi[:, m, k, :], rhs,
                                         start=(k == 0), stop=False)
                    nc.tensor.matmul(pst[:], gib[0:1, m * 128:(m + 1) * 128],
                                     sb["onesT"][0:1, :], start=False, stop=True)
                    nc.vector.tensor_copy(gi[:, m * T:(m + 1) * T], pst[:])

                wh = w4("encWh%d" % lay)
                bhn = sb["enc_bhn%d" % lay]
                gR = gi.rearrange("p (m t) -> p m t", t=T)
                hsl = hT[:, lay * 4:(lay + 1) * 4]
                for t in range(T):
                    hT16 = wk.tile([128, 4], F16, tag="hT16")
                    nc.vector.tensor_copy(hT16[:], hsl)
                    pg = ps.tile([128, 12], F32, tag="pB")
                    for m in range(12):
                        for k in range(4):
                            nc.tensor.matmul(pg[:, m:m + 1], wh[:, m, k, :],
                                             hT16[:, k:k + 1],
                                             start=(k == 0), stop=(k == 3))
                    trz = wk.tile([128, 8], F32, tag="trz")
                    nc.vector.tensor_add(trz[:], pg[:, 0:8], gR[:, 0:8, t])
                    srz = wk.tile([128, 8], F32, tag="srz")
                    nc.scalar.activation(srz[:], trz[:], AF.Sigmoid)
                    hnb = wk.tile([128, 4], F32, tag="hnb")
                    nc.vector.tensor_add(hnb[:], pg[:, 8:12], bhn[:])
                    nc.vector.tensor_mul(hnb[:], srz[:, 0:4], hnb[:])
                    nc.vector.tensor_add(hnb[:], hnb[:], gR[:, 8:12, t])
                    n_t = wk.tile([128, 4], F32, tag="n_t")
                    nc.scalar.activation(n_t[:], hnb[:], AF.Tanh)
                    d_t = wk.tile([128, 4], F32, tag="d_t")
                    nc.vector.tensor_sub(d_t[:], hsl, n_t[:])
                    nc.vector.tensor_mul(d_t[:], srz[:, 4:8], d_t[:])
                    nc.vector.tensor_add(hsl, n_t[:], d_t[:])
                    dst = (l0OT16 if lay == 0 else encOT16)
                    nc.vector.tensor_copy(
                        dst.rearrange("p (k t) -> p k t", k=4)[:, :, t], hsl)
            for k in range(4):
                pt = ps.tile([T, 128], F16, tag="pmid")
                nc.tensor.transpose(
                    pt[:], encOT16.rearrange("p (k t) -> p k t", k=4)[:, k, :],
                    sb["ident16"][:])
                nc.vector.tensor_copy(encNat[:, k * 128:(k + 1) * 128], pt[:])

            for name, tg in [("decWi0", "wbig0"), ("decWh0", "wbig1"),
                             ("decWi1", "wbig2"), ("decWh1", "wbig3")]:
                sb[name] = wp.tile([128, 12 * 4 * 128], F16, tag=tg, name=name)
                nc.sync.dma_start(out=sb[name][:], in_=din[name][:])

            # decoder hidden init: broadcast hT cols to 5 beams
            for c in range(8):
                nc.vector.tensor_copy(hdT[:, c * 5:(c + 1) * 5],
                                      hT[:, c:c + 1].broadcast_to([128, 5]))

            WoutR = sb["WoutT"].rearrange("p (k v) -> p k v", k=4)
            Wc4 = sb["WcT"].rearrange("p (m k c) -> p m k c", k=8, c=128)

            # =============== DECODER ===============
            for st in range(S):
                nc.vector.tensor_copy(hdT16[:], hdT[:])
                lin16 = xdT
                for lay in range(2):
                    wi = w4("decWi%d" % lay)
                    wh = w4("decWh%d" % lay)
                    brz = sb["dec_brz%d" % lay]
                    bin_ = sb["dec_bin%d" % lay]
                    bhn = sb["dec_bhn%d" % lay]
                    hsl = hdT[:, lay * 20:(lay + 1) * 20]
                    h16 = hdT16[:, lay * 20:(lay + 1) * 20]
                    prz = ps.tile([128, 40], F32, tag="prz")
                    pin = ps.tile([128, 20], F32, tag="pin")
                    phn = ps.tile([128, 20], F32, tag="phn")
                    for m in range(12):
                        if m < 8:
                            dst = prz[:, m * 5:(m + 1) * 5]
                        else:
                            dst = pin[:, (m - 8) * 5:(m - 7) * 5]
                        for k in range(4):
                            nc.tensor.matmul(dst, wi[:, m, k, :],
                                             lin16[:, k * 5:(k + 1) * 5],
                                             start=(k == 0), stop=False)
                        if m < 8:
                            for k in range(4):
                                nc.tensor.matmul(dst, wh[:, m, k, :],
                                                 h16[:, k * 5:(k + 1) * 5],
                                                 start=False, stop=False)
                            nc.tensor.matmul(dst, brz[0:1, m * 128:(m + 1) * 128],
                                             sb["ones15"][0:1, :], start=False, stop=True)
                        else:
                            nc.tensor.matmul(dst, bin_[0:1, (m - 8) * 128:(m - 7) * 128],
                                             sb["ones15"][0:1, :], start=False, stop=True)
                            dstn = phn[:, (m - 8) * 5:(m - 7) * 5]
                            for k in range(4):
                                nc.tensor.matmul(dstn, wh[:, m, k, :],
                                                 h16[:, k * 5:(k + 1) * 5],
                                                 start=(k == 0), stop=False)
                            nc.tensor.matmul(dstn, bhn[0:1, (m - 8) * 128:(m - 7) * 128],
                                             sb["ones15"][0:1, :], start=False, stop=True)
                    srz = wk.tile([128, 40], F32, tag="dsrz")
                    nc.scalar.activation(srz[:], prz[:], AF.Sigmoid)
                    tn = wk.tile([128, 20], F32, tag="dtn")
                    nc.vector.tensor_mul(tn[:], srz[:, 0:20], phn[:])
                    nc.vector.tensor_add(tn[:], tn[:], pin[:])
                    n_t = wk.tile([128, 20], F32, tag="dn")
                    nc.scalar.activation(n_t[:], tn[:], AF.Tanh)
                    d_t = wk.tile([128, 20], F32, tag="dd")
                    nc.vector.tensor_sub(d_t[:], hsl, n_t[:])
                    nc.vector.tensor_mul(d_t[:], srz[:, 20:40], d_t[:])
                    nc.vector.tensor_add(hsl, n_t[:], d_t[:])
                    nh16 = wk.tile([128, 20], F16, tag="nh16")
                    nc.vector.tensor_copy(nh16[:], hsl)
                    lin16 = nh16
                rnn16 = lin16

                # ---- attention ----
                pe_ = ps.tile([16, 128], F32, tag="pmid")
                r4 = rnn16.rearrange("p (k b) -> p k b", k=4)
                e4 = encOT16.rearrange("p (k t) -> p k t", k=4)
                for k in range(4):
                    nc.tensor.matmul(pe_[0:5, 0:T], r4[:, k, :], e4[:, k, :],
                                     start=(k == 0), stop=(k == 3))
                mx = wk.tile([16, 1], F32, tag="amx")
                nc.vector.tensor_reduce(mx[0:5, :], pe_[0:5, 0:T],
                                        axis=mybir.AxisListType.X, op=ALU.max,
                                        negate=True)
                aexp = wk.tile([16, 128], F32, tag="aexp")
                asum = wk.tile([16, 1], F32, tag="asum")
                nc.scalar.activation(aexp[0:5, 0:T], pe_[0:5, 0:T], AF.Exp,
                                     bias=mx[0:5, :], accum_out=asum[0:5, :])
                ars = wk.tile([16, 1], F32, tag="ars")
                nc.vector.reciprocal(ars[0:5, :], asum[0:5, :])
                nc.vector.tensor_scalar_mul(aexp[0:5, 0:T], aexp[0:5, 0:T],
                                            ars[0:5, :])
                pat = ps.tile([128, 8], F32, tag="pmid")
                nc.tensor.transpose(pat[:T, 0:5], aexp[0:5, 0:T], ident[0:5, 0:5])
                aT16 = wk.tile([128, 5], F16, tag="aT16")
                nc.vector.tensor_copy(aT16[0:T, :], pat[0:T, 0:5])
                pctx = ps.tile([128, 20], F32, tag="prz")
                for m in range(4):
                    nc.tensor.matmul(pctx[:, m * 5:(m + 1) * 5],
                                     encNat[:, m * 128:(m + 1) * 128],
                                     aT16[0:T, :], start=True, stop=True)
                ctx16 = wk.tile([128, 20], F16, tag="ctx16")
                nc.vector.tensor_copy(ctx16[:], pctx[:])

                # ---- Wc + tanh ----
                pcc = ps.tile([128, 20], F32, tag="pin")
                for m in range(4):
                    dst = pcc[:, m * 5:(m + 1) * 5]
                    for k in range(8):
                        rhs = rnn16[:, (k % 4) * 5:(k % 4) * 5 + 5] if k < 4 else \
                            ctx16[:, (k - 4) * 5:(k - 3) * 5]
                        nc.tensor.matmul(dst, Wc4[:, m, k, :], rhs,
                                         start=(k == 0), stop=False)
                    nc.tensor.matmul(dst, sb["bcr"][0:1, m * 128:(m + 1) * 128],
                                     sb["ones15"][0:1, :], start=False, stop=True)
                ccT = wk.tile([128, 20], F32, tag="ccT")
                nc.scalar.activation(ccT[:], pcc[:], AF.Tanh)

                # ---- Wout (fp32, vocab shard) ----
                pA = ps.tile([128, 500], F32, tag="pA")
                pB = ps.tile([128, 500], F32, tag="pB")
                for k in range(4):
                    for c in range(8):
                        pt_ = pA if c < 4 else pB
                        dst = pt_[32 * (c % 4):32 * (c % 4) + 5, :]
                        nc.tensor.matmul(dst, ccT[:, k * 5:(k + 1) * 5],
                                         WoutR[:, k, c * 500:(c + 1) * 500],
                                         start=(k == 0), stop=(k == 3),
                                         tile_position=(0, 32 * (c % 4)))
                nc.vector.tensor_add(pA[:], pA[:], sb["boutRep"][:, 0:500])
                nc.vector.tensor_add(pB[:], pB[:], sb["boutRep"][:, 500:1000])

                eA = wk.tile([128, 500], F32, tag="eA")
                sA = wk.tile([128, 1], F32, tag="sA")
                nc.scalar.activation(eA[:], pA[:], AF.Exp, accum_out=sA[:])
                eB = wk.tile([128, 500], F32, tag="eB")
                sB = wk.tile([128, 1], F32, tag="sB")
                nc.scalar.activation(eB[:], pB[:], AF.Exp, accum_out=sB[:])
                rA = wk.tile([128, 500], F32, tag="rA")
                nc.vector.tensor_copy(rA[:], pA[:])
                rB = wk.tile([128, 500], F32, tag="rB")
                nc.vector.tensor_copy(rB[:], pB[:])

                v8A = wk.tile([128, 8], F32, tag="v8A")
                i8A = wk.tile([128, 8], U32, tag="i8A")
                nc.vector.max(v8A[:], rA[:])
                nc.vector.max_index(i8A[:], v8A[:], rA[:])
                v8B = wk.tile([128, 8], F32, tag="v8B")
                i8B = wk.tile([128, 8], U32, tag="i8B")
                nc.vector.max(v8B[:], rB[:])
                nc.vector.max_index(i8B[:], v8B[:], rB[:])
                gfA = wk.tile([128, 8], F32, tag="gfA")
                nc.vector.tensor_copy(gfA[:], i8A[:])
                nc.vector.tensor_scalar_add(gfA[:], gfA[:], sb["voffA"][:])
                gfB = wk.tile([128, 8], F32, tag="gfB")
                nc.vector.tensor_copy(gfB[:], i8B[:])
                nc.vector.tensor_scalar_add(gfB[:], gfB[:], sb["voffB"][:])

                # pre-AG merge -> [5, 129]
                pm = ps.tile([16, 136], F32, tag="pmid")
                for c in range(4):
                    nc.tensor.matmul(pm[0:5, c * 8:(c + 1) * 8],
                                     sb["Icc"][:, c * 5:(c + 1) * 5], v8A[:],
                                     start=True, stop=True)
                    nc.tensor.matmul(pm[0:5, 32 + c * 8:32 + (c + 1) * 8],
                                     sb["Icc"][:, c * 5:(c + 1) * 5], v8B[:],
                                     start=True, stop=True)
                    nc.tensor.matmul(pm[0:5, 64 + c * 8:64 + (c + 1) * 8],
                                     sb["Icc"][:, c * 5:(c + 1) * 5], gfA[:],
                                     start=True, stop=True)
                    nc.tensor.matmul(pm[0:5, 96 + c * 8:96 + (c + 1) * 8],
                                     sb["Icc"][:, c * 5:(c + 1) * 5], gfB[:],
                                     start=True, stop=True)
                nc.tensor.matmul(pm[0:5, 128:129], sb["Isum"][:], sA[:],
                                 start=True, stop=False)
                nc.tensor.matmul(pm[0:5, 128:129], sb["Isum"][:], sB[:],
                                 start=False, stop=True)
                nc.vector.tensor_copy(payload[:, 0:129], pm[0:5, 0:129])

                # AllGather
                NOCC = bool(os.environ.get("K_NOCC"))
                agi = dramp.tile([5, 136], F32, tag="agi")
                ago = dramp.tile([NCORE * 5, 136], F32, tag="ago",
                                 addr_space="Shared")
                nc.gpsimd.dma_start(agi[:], payload[:])
                if NOCC:
                    nc.gpsimd.dma_start(ago[0:5, :], agi[:])
                else:
                    nc.gpsimd.collective_compute(
                        "AllGather", ALU.bypass,
                        ins=[agi.opt()], outs=[ago.opt()],
                        replica_groups=[list(range(NCORE))])
                nc.gpsimd.dma_start(gsb[:], ago[:])

                # post-AG merge: per-beam 512 candidates
                pqv = ps.tile([16, 512], F32, tag="pA")
                pqg = ps.tile([16, 512], F32, tag="pB")
                pqs = ps.tile([16, 1], F32, tag="pqs")
                for r in range(NCORE):
                    nc.tensor.matmul(pqv[0:5, r * 64:(r + 1) * 64],
                                     sb["Ir"][:, r * 5:(r + 1) * 5],
                                     gsb[:, 0:64], start=True, stop=True)
                    nc.tensor.matmul(pqg[0:5, r * 64:(r + 1) * 64],
                                     sb["Ir"][:, r * 5:(r + 1) * 5],
                                     gsb[:, 64:128], start=True, stop=True)
                nc.tensor.matmul(pqs[0:5, :], sb["IrSum"][:], gsb[:, 128:129],
                                 start=True, stop=True)
                nc.vector.tensor_copy(mv[0:5, 0:512], pqv[0:5, :])
                nc.vector.tensor_copy(mv[0:5, 512:1024], pqg[0:5, :])
                nc.scalar.activation(lse[0:5, :], pqs[0:5, :], AF.Ln)

                cfull = wk.tile([16, 512], F32, tag="cfull")
                nc.vector.tensor_scalar_sub(cfull[:], mv[:, 0:512], lse[:])
                nc.vector.tensor_scalar_add(cfull[:], cfull[:], scores[:])
                for b in range(5):
                    pfl = ps.tile([16, 512], F32, tag="pqs")
                    nc.tensor.matmul(pfl[0:1, :], sb["Ib"][:, b:b + 1],
                                     cfull[:], start=True, stop=True)
                    nc.vector.tensor_copy(cflat[0:1, b * 512:(b + 1) * 512],
                                          pfl[0:1, :])
                    pfl2 = ps.tile([16, 512], F32, tag="pfg")
                    nc.tensor.matmul(pfl2[0:1, :], sb["Ib"][:, b:b + 1],
                                     mv[:, 512:1024], start=True, stop=True)
                    nc.vector.tensor_copy(gflat[0:1, b * 512:(b + 1) * 512],
                                          pfl2[0:1, :])
                nc.sync.dma_start(out=gdram[:], in_=gflat[:])

                ft = wk.tile([1, 8], F32, tag="ft")
                nc.vector.max(ft[:], cflat[:])
                fp_ = wk.tile([1, 8], U32, tag="fp_")
                nc.vector.max_index(fp_[:], ft[:], cflat[:])
                parw = wk.tile([1, 8], U32, tag="parw")
                nc.vector.tensor_scalar(parw[:], fp_[:], 9, None,
                                        ALU.logical_shift_right)
                parf = wk.tile([1, 8], F32, tag="parf")
                nc.vector.tensor_copy(parf[:], parw[:])

                # scores + parents to columns via transpose
                nc.vector.tensor_copy(tt10[0:1, 0:5], ft[0:1, 0:5])
                nc.vector.tensor_copy(tt10[0:1, 5:10], parf[0:1, 0:5])
                ptr = ps.tile([16, 8], F32, tag="pqs")
                nc.tensor.transpose(ptr[0:5, 0:1], tt10[0:1, 0:5], ident[0:1, 0:1])
                nc.tensor.transpose(ptr[0:5, 1:2], tt10[0:1, 5:10], ident[0:1, 0:1])
                nc.vector.tensor_copy(scores[0:5, :], ptr[0:5, 0:1])
                nc.vector.tensor_copy(parsAcc[:, st:st + 1], ptr[:, 1:2])
                # winner positions -> column, then vocab ids via indirect DMA
                pposf = wk.tile([1, 8], F32, tag="pposf")
                nc.vector.tensor_copy(pposf[:], fp_[:])
                ptr2 = ps.tile([16, 8], F32, tag="pfg")
                nc.tensor.transpose(ptr2[0:5, 0:1], pposf[0:1, 0:5],
                                    ident[0:1, 0:1])
                posc = wk.tile([16, 1], I32, tag="posc")
                nc.gpsimd.memset(posc[:], 0)
                nc.vector.tensor_copy(posc[0:5, :], ptr2[0:5, 0:1])
                tokc = wk.tile([16, 1], F32, tag="tokc")
                nc.gpsimd.indirect_dma_start(
                    out=tokc[:], out_offset=None, in_=gdram[:],
                    in_offset=bass.IndirectOffsetOnAxis(ap=posc[:, 0:1], axis=0),
                    bounds_check=5 * 512 - 1, oob_is_err=False)
                nc.vector.tensor_copy(toksAcc[:, st:st + 1], tokc[:])
                idxf = wk.tile([16, 1], F32, tag="idxf")
                nc.vector.tensor_copy(idxf[:], tokc[:])
                nc.vector.tensor_copy(idxt[0:5, :], idxf[0:5, :])

                if st == S - 1:
                    break

                # embedding gather for new tokens
                pemb = wk.tile([16, 512], F32, tag="pemb")
                if os.environ.get("K_NOIDMA"):
                    nc.sync.dma_start(out=pemb[:], in_=din["emb"][0:16, :])
                else:
                    nc.gpsimd.indirect_dma_start(
                        out=pemb[:], out_offset=None, in_=din["emb"][:],
                        in_offset=bass.IndirectOffsetOnAxis(ap=idxt[:, 0:1], axis=0),
                        bounds_check=V - 1, oob_is_err=False)
                pxt = ps.tile([128, 20], F32, tag="pmid")
                for k in range(4):
                    nc.tensor.transpose(pxt[:, k * 5:(k + 1) * 5],
                                        pemb[0:5, k * 128:(k + 1) * 128],
                                        ident[0:5, 0:5])
                nc.vector.tensor_copy(xdT[:], pxt[:])

                # hidden reorder by parent (transpose -> permute-matmul -> transpose)
                pth = ps.tile([64, 128], F32, tag="prz")
                nc.tensor.transpose(pth[0:40, :], hdT[:], ident[:])
                th = wk.tile([64, 128], F32, tag="th")
                nc.vector.tensor_copy(th[0:40, :], pth[0:40, :])
                tgt = wk.tile([1, 40], F32, tag="tgt")
                parv = parf[0:1, 0:5].unsqueeze(1).broadcast_to([1, 8, 5])
                nc.vector.tensor_tensor(tgt[0:1, :].rearrange("a (c b) -> a c b", b=5),
                                        sb["base40"][0:1, :].rearrange("a (c b) -> a c b", b=5),
                                        parv, op=ALU.add)
                ptgt = ps.tile([64, 40], F32, tag="pin")
                nc.tensor.matmul(ptgt[0:40, :], sb["ones40"][0:1, 0:40],
                                 tgt[0:1, :], start=True, stop=True)
                P40 = wk.tile([64, 40], F32, tag="P40")
                nc.vector.tensor_tensor(
                    P40[0:40, :],
                    sb["rowidx40"][0:40, 0:1].broadcast_to([40, 40]),
                    ptgt[0:40, :],
                    op=ALU.is_equal)
                pnh = ps.tile([64, 128], F32, tag="prz")
                nc.tensor.matmul(pnh[0:40, :], P40[0:40, :], th[0:40, :],
                                 start=True, stop=True)
                th2 = wk.tile([64, 128], F32, tag="th2")
                nc.vector.tensor_copy(th2[0:40, :], pnh[0:40, :])
                ptb = ps.tile([128, 40], F32, tag="pmid")
                nc.tensor.transpose(ptb[:], th2[0:40, :], ident[0:40, 0:40])
                nc.vector.tensor_copy(hdT[:], ptb[:])

            nc.sync.dma_start(
                out=out_tp[0:1, :].rearrange("a (s b) -> (a b) s", b=5),
                in_=toksAcc[0:5, :])
            nc.sync.dma_start(
                out=out_tp[1:2, :].rearrange("a (s b) -> (a b) s", b=5),
                in_=parsAcc[0:5, :])
            nc.sync.dma_start(out=out_sc[:], in_=scores[0:5, :])
    nc.compile()
    return nc


def _prep_inputs(inputs, T, S):
    f32 = np.float32
    emb = np.asarray(inputs["emb"], f32)
    seq = np.asarray(inputs["input_seq"]).reshape(-1).astype(np.int64)
    sos = int(np.asarray(inputs["SOS_token"]))
    g = {}
    for nm in ["enc_Wi", "enc_Wh", "enc_bi", "enc_bh",
               "dec_Wi", "dec_Wh", "dec_bi", "dec_bh"]:
        g[nm] = np.asarray(inputs[nm], f32)
    Wc = np.asarray(inputs["Wc"], f32)
    bc = np.asarray(inputs["bc"], f32)
    Wout = np.asarray(inputs["Wout"], f32)
    bout = np.asarray(inputs["bout"], f32)

    d = {}
    for lay in range(2):
        d[f"encWh{lay}"] = _tile_wT(g["enc_Wh"][lay]).astype(np.float16)
        d[f"encWi{lay}"] = _tile_wT(g["enc_Wi"][lay]).astype(np.float16)
        d[f"decWi{lay}"] = _tile_wT(g["dec_Wi"][lay]).astype(np.float16)
        d[f"decWh{lay}"] = _tile_wT(g["dec_Wh"][lay]).astype(np.float16)
    for lay in range(2):
        bi, bh = g["enc_bi"][lay], g["enc_bh"][lay]
        bb = bi.copy()
        bb[:1024] += bh[:1024]
        d[f"enc_gib{lay}"] = bb.reshape(1, -1)
        d[f"enc_bhn{lay}"] = bh[1024:].reshape(4, 128).T.copy()
        dbi, dbh = g["dec_bi"][lay], g["dec_bh"][lay]
        d[f"dec_brz{lay}"] = (dbi[:1024] + dbh[:1024]).reshape(1, -1)
        d[f"dec_bin{lay}"] = dbi[1024:].reshape(1, -1)
        d[f"dec_bhn{lay}"] = dbh[1024:].reshape(1, -1)
    d["WcT"] = _tile_wT(Wc).astype(np.float16)
    d["bcr"] = bc.reshape(1, -1)
    d["ident"] = np.eye(128, dtype=f32)
    d["ident16"] = np.eye(128, dtype=np.float16)
    d["ones15"] = np.ones((1, 5), f32)
    d["onesT"] = np.ones((1, T), f32)
    x = emb[seq]  # [T, H]
    d["xT"] = np.ascontiguousarray(
        x.T.reshape(4, 128, T).transpose(1, 0, 2).reshape(128, 4 * T)
    ).astype(np.float16)
    xd = np.broadcast_to(emb[sos], (5, 512)).T  # [512,5]
    d["xd0"] = np.ascontiguousarray(
        xd.reshape(4, 128, 5).transpose(1, 0, 2).reshape(128, 20)).astype(np.float16)
    Icc = np.zeros((128, 20), f32)
    for c in range(4):
        for b in range(5):
            Icc[32 * c + b, c * 5 + b] = 1.0
    d["Icc"] = Icc
    d["Isum"] = Icc.reshape(128, 4, 5).sum(1)
    Ir = np.zeros((40, 40), f32)
    for r in range(8):
        for b in range(5):
            Ir[5 * r + b, r * 5 + b] = 1.0
    d["Ir"] = Ir
    d["IrSum"] = Ir.reshape(40, 8, 5).sum(1)
    Ib = np.zeros((16, 5), f32)
    for b in range(5):
        Ib[b, b] = 1.0
    d["Ib"] = Ib
    d["base40"] = (np.arange(40) // 5 * 5).astype(f32).reshape(1, 40)
    d["rowidx40"] = np.arange(64, dtype=f32).reshape(64, 1)
    d["ones40"] = np.ones((1, 40), f32)
    h0 = np.zeros((16, 1), f32)
    h0[1:5] = NEG
    d["h0sc"] = h0
    d["emb"] = emb

    # per-core tensors
    per_core = []
    WoutT = Wout.T  # [H, V]
    for r in range(NCORE):
        dc = dict(d)
        sh = WoutT[:, r * VS:(r + 1) * VS]  # [512, 4000]
        dc["WoutT"] = np.ascontiguousarray(
            sh.reshape(4, 128, VS).transpose(1, 0, 2).reshape(128, 4 * VS))
        bsh = bout[r * VS:(r + 1) * VS]
        br = np.zeros((128, 1000), f32)
        for c in range(8):
            half = c // 4
            for b in range(5):
                br[32 * (c % 4) + b, half * 500:(half + 1) * 500] = bsh[c * 500:(c + 1) * 500]
        dc["boutRep"] = br
        voffA = np.zeros((128, 1), f32)
        voffB = np.zeros((128, 1), f32)
        for c in range(4):
            voffA[32 * c:32 * (c + 1)] = r * VS + c * 500
            voffB[32 * c:32 * (c + 1)] = r * VS + (c + 4) * 500
        dc["voffA"] = voffA
        dc["voffB"] = voffB
        per_core.append(dc)
    return per_core


def kernel(**inputs):
    T = int(np.asarray(inputs["input_length"]))
    S = int(np.asarray(inputs["max_length"]))
    key = (T, S)
    if key not in _CACHE:
        _CACHE[key] = build(T, S)
    nc = _CACHE[key]
    in_maps = _prep_inputs(inputs, T, S)
    res = run_bass_kernel_spmd(nc, in_maps, list(range(NCORE)))
    r0 = res.results[0]
    toks = r0["out_tp"][0].reshape(S, 5).astype(np.int64)
    pars = r0["out_tp"][1].reshape(S, 5).astype(np.int64)
    scores = r0["out_sc"].reshape(5)
    best = int(np.argmax(scores))
    seq = np.zeros(S, np.int64)
    b = best
    for t in range(S - 1, -1, -1):
        seq[t] = toks[t, b]
        b = pars[t, b]
    in_seq = np.asarray(inputs["input_seq"])
    seq = seq.astype(in_seq.dtype) if np.issubdtype(in_seq.dtype, np.integer) else seq.astype(np.int32)
    return seq, np.float32(scores[best])


# revision 20
# speedup vs baseline: 1.3692x; 1.3692x over previous
"""Trainium2 Bass kernel: GRU encoder + beam-search decoder (V=32000,H=512,L=2,BW=5).

All 8 cores run an identical SPMD program. Encoder + decoder GRU/attention/Wc
replicated per core in fp16 weights/activations with fp32 accumulation
(host-verified to reproduce the reference token sequence). Output projection is
vocab-sharded (4000/core) in fp32; per-core per-beam top-8 candidates and
exp-sum partials are AllGathered every step and each core performs the
identical merge. Backtrack runs on host.
"""
import os
import sys
import numpy as np

sys.path.insert(0, "/opt/trn_rl_repo")

import concourse.bass as bass
import concourse.bacc as bacc
import concourse.mybir as mybir
from concourse.tile import TileContext
from concourse.bass_utils import run_bass_kernel_spmd

F32 = mybir.dt.float32
F16 = mybir.dt.float16
U32 = mybir.dt.uint32
U16 = mybir.dt.uint16
I32 = mybir.dt.int32
AF = mybir.ActivationFunctionType
ALU = mybir.AluOpType

V, H, L, BW = 32000, 512, 2, 5
NCORE = 8
VS = V // NCORE
NEG = -1e9

_CACHE = {}


def _tile_wT(W):
    """W: [G,K] fp32 (g = W @ x). -> [128, MC*KC*128] lhsT tile layout:
    tile (m,k) at cols (m*KC+k)*128 ; value[p, c] = W[128m+c, 128k+p]."""
    G, K = W.shape
    MC, KC = G // 128, K // 128
    out = np.zeros((128, MC * KC * 128), np.float32)
    for m in range(MC):
        for k in range(KC):
            blk = W[m * 128:(m + 1) * 128, k * 128:(k + 1) * 128]
            out[:, (m * KC + k) * 128:(m * KC + k + 1) * 128] = blk.T
    return out


def build(T, S):
    nc = bacc.Bacc(None, target_bir_lowering=False)
    din = {}

    def dram_in(name, shape, dt=F32):
        din[name] = nc.dram_tensor(name, shape, dt, kind="ExternalInput")
        return din[name]

    for nm in ["encWh0", "encWh1", "encWi0", "encWi1",
               "decWi0", "decWh0", "decWi1", "decWh1"]:
        dram_in(nm, [128, 12 * 4 * 128], F16)
    dram_in("WcT", [128, 4 * 8 * 128], F16)
    dram_in("WoutT", [128, 4 * VS], F32)
    dram_in("xT", [128, 4 * T], F16)
    dram_in("enc_gib0", [1, 12 * 128])
    dram_in("enc_gib1", [1, 12 * 128])
    dram_in("enc_bhn0", [128, 4])
    dram_in("enc_bhn1", [128, 4])
    for nm in ["dec_brz0", "dec_brz1"]:
        dram_in(nm, [1, 8 * 128])
    for nm in ["dec_bin0", "dec_bin1", "dec_bhn0", "dec_bhn1", "bcr"]:
        dram_in(nm, [1, 4 * 128])
    dram_in("boutRep", [128, 2 * 500])
    dram_in("ident", [128, 128])
    dram_in("ident16", [128, 128], F16)
    dram_in("ones15", [1, 5])
    dram_in("onesT", [1, T])
    dram_in("Icc", [128, 20])
    dram_in("Isum", [128, 5])
    dram_in("Ir", [40, 40])
    dram_in("IrSum", [40, 5])
    dram_in("Ib", [16, 5])
    dram_in("voffA", [128, 1])
    dram_in("voffB", [128, 1])
    dram_in("base40", [1, 40])
    dram_in("rowidx40", [64, 1])
    dram_in("ones40", [1, 40])
    dram_in("h0sc", [16, 1])
    dram_in("xd0", [128, 4 * 5], F16)
    dram_in("emb", [V, H], F16)

    out_tp = nc.dram_tensor("out_tp", [2, S * 5], F32, kind="ExternalOutput")
    gdram = nc.dram_tensor("gdram", [5 * 512, 1], F32)
    out_sc = nc.dram_tensor("out_sc", [5, 1], F32, kind="ExternalOutput")


    with TileContext(nc) as tc:
        with tc.tile_pool(name="w", bufs=1) as wp, \
             tc.tile_pool(name="wk", bufs=1) as wk, \
             tc.tile_pool(name="dramp", bufs=2, space="DRAM") as dramp, \
             tc.tile_pool(name="ps", bufs=1, space="PSUM") as ps:

            sb = {}
            DEFER = ("decWi0", "decWh0", "decWi1", "decWh1", "emb")
            WTAG = {"encWi0": "wbig0", "encWh0": "wbig1",
                    "encWi1": "wbig2", "encWh1": "wbig3"}
            for name, t in din.items():
                if name in DEFER:
                    continue
                sb[name] = wp.tile(list(t.shape), t.dtype,
                                   tag=WTAG.get(name, name), name=name)
                nc.sync.dma_start(out=sb[name][:], in_=t[:])

            def w4(name):
                return sb[name].rearrange("p (m k c) -> p m k c", k=4, c=128)

            ident = sb["ident"]

            hT = wp.tile([128, 2 * 4], F32, tag="hT")
            l0OT16 = wp.tile([128, 4 * T], F16, tag="l0OT16")
            encOT16 = wp.tile([128, 4 * T], F16, tag="encOT16")
            encNat = wp.tile([T, 4 * 128], F16, tag="encNat")
            gi = wp.tile([128, 12 * T], F32, tag="gi")
            hdT = wp.tile([128, 2 * 4 * 5], F32, tag="hdT")
            hdT16 = wp.tile([128, 2 * 4 * 5], F16, tag="hdT16")
            xdT = wp.tile([128, 4 * 5], F16, tag="xdT")
            scores = wp.tile([16, 1], F32, tag="scores")
            lse = wp.tile([16, 1], F32, tag="lse")
            toksAcc = wp.tile([16, S], F32, tag="toksAcc")
            parsAcc = wp.tile([16, S], F32, tag="parsAcc")
            cflat = wp.tile([1, 5 * 512], F32, tag="cflat")
            gflat = wp.tile([1, 5 * 512], F32, tag="gflat")
            mv = wp.tile([16, 1024], F32, tag="mv")
            gsb = wp.tile([40, 136], F32, tag="gsb")
            payload = wp.tile([5, 136], F32, tag="payload")
            idxt = wp.tile([16, 1], I32, tag="idxt")
            tt10 = wp.tile([1, 16], F32, tag="tt10")

            nc.vector.tensor_copy(scores[:], sb["h0sc"][:])
            nc.gpsimd.memset(hT[:], 0.0)
            nc.gpsimd.memset(mv[:], NEG)
            nc.gpsimd.memset(idxt[:], 0)
            nc.gpsimd.memset(lse[:], 0.0)
            nc.vector.tensor_copy(xdT[:], sb["xd0"][:])

            # =============== ENCODER ===============
            xTr = sb["xT"].rearrange("p (k t) -> p k t", k=4)
            for lay in range(2):
                wi = w4("encWi%d" % lay)
                gib = sb["enc_gib%d" % lay]
                for m in range(12):
                    pst = ps.tile([128, T], F32, tag="pA")
                    for k in range(4):
                        rhs = xTr[:, k, :] if lay == 0 else \
                            l0OT16.rearrange("p (k t) -> p k t", k=4)[:, k, :]
                        nc.tensor.matmul(pst[:], wi[:, m, k, :], rhs,
                                         start=(k == 0), stop=False)
                    nc.tensor.matmul(pst[:], gib[0:1, m * 128:(m + 1) * 128],
                                     sb["onesT"][0:1, :], start=False, stop=True)
                    nc.vector.tensor_copy(gi[:, m * T:(m + 1) * T], pst[:])

                wh = w4("encWh%d" % lay)
                bhn = sb["enc_bhn%d" % lay]
                gR = gi.rearrange("p (m t) -> p m t", t=T)
                hsl = hT[:, lay * 4:(lay + 1) * 4]
                for t in range(T):
                    hT16 = wk.tile([128, 4], F16, tag="hT16")
                    nc.vector.tensor_copy(hT16[:], hsl)
                    pg = ps.tile([128, 12], F32, tag="pB")
                    for m in range(12):
                        for k in range(4):
                            nc.tensor.matmul(pg[:, m:m + 1], wh[:, m, k, :],
                                             hT16[:, k:k + 1],
                                             start=(k == 0), stop=(k == 3))
                    trz = wk.tile([128, 8], F32, tag="trz")
                    nc.vector.tensor_add(trz[:], pg[:, 0:8], gR[:, 0:8, t])
                    srz = wk.tile([128, 8], F32, tag="srz")
                    nc.scalar.activation(srz[:], trz[:], AF.Sigmoid)
                    hnb = wk.tile([128, 4], F32, tag="hnb")
                    nc.vector.tensor_add(hnb[:], pg[:, 8:12], bhn[:])
                    nc.vector.tensor_mul(hnb[:], srz[:, 0:4], hnb[:])
                    nc.vector.tensor_add(hnb[:], hnb[:], gR[:, 8:12, t])
                    n_t = wk.tile([128, 4], F32, tag="n_t")
                    nc.scalar.activation(n_t[:], hnb[:], AF.Tanh)
                    d_t = wk.tile([128, 4], F32, tag="d_t")
                    nc.vector.tensor_sub(d_t[:], hsl, n_t[:])
                    nc.vector.tensor_mul(d_t[:], srz[:, 4:8], d_t[:])
                    nc.vector.tensor_add(hsl, n_t[:], d_t[:])
                    dst = (l0OT16 if lay == 0 else encOT16)
                    nc.vector.tensor_copy(
                        dst.rearrange("p (k t) -> p k t", k=4)[:, :, t], hsl)
            for k in range(4):
                pt = ps.tile([T, 128], F16, tag="pmid")
                nc.tensor.transpose(
                    pt[:], encOT16.rearrange("p (k t) -> p k t", k=4)[:, k, :],
                    sb["ident16"][:])
                nc.vector.tensor_copy(encNat[:, k * 128:(k + 1) * 128], pt[:])

            for name, tg in [("decWi0", "wbig0"), ("decWh0", "wbig1"),
                             ("decWi1", "wbig2"), ("decWh1", "wbig3")]:
                sb[name] = wp.tile([128, 12 * 4 * 128], F16, tag=tg, name=name)
                nc.sync.dma_start(out=sb[name][:], in_=din[name][:])

            # decoder hidden init: broadcast hT cols to 5 beams
            for c in range(8):
                nc.vector.tensor_copy(hdT[:, c * 5:(c + 1) * 5],
                                      hT[:, c:c + 1].broadcast_to([128, 5]))

            WoutR = sb["WoutT"].rearrange("p (k v) -> p k v", k=4)
            Wc4 = sb["WcT"].rearrange("p (m k c) -> p m k c", k=8, c=128)

            # =============== DECODER ===============
            for st in range(S):
                nc.vector.tensor_copy(hdT16[:], hdT[:])
                lin16 = xdT
                for lay in range(2):
                    wi = w4("decWi%d" % lay)
                    wh = w4("decWh%d" % lay)
                    brz = sb["dec_brz%d" % lay]
                    bin_ = sb["dec_bin%d" % lay]
                    bhn = sb["dec_bhn%d" % lay]
                    hsl = hdT[:, lay * 20:(lay + 1) * 20]
                    h16 = hdT16[:, lay * 20:(lay + 1) * 20]
                    prz = ps.tile([128, 40], F32, tag="prz")
                    pin = ps.tile([128, 20], F32, tag="pin")
                    phn = ps.tile([128, 20], F32, tag="phn")
                    for m in range(12):
                        if m < 8:
                            dst = prz[:, m * 5:(m + 1) * 5]
                        else:
                            dst = pin[:, (m - 8) * 5:(m - 7) * 5]
                        for k in range(4):
                            nc.tensor.matmul(dst, wi[:, m, k, :],
                                             lin16[:, k * 5:(k + 1) * 5],
                                             start=(k == 0), stop=False)
                        if m < 8:
                            for k in range(4):
                                nc.tensor.matmul(dst, wh[:, m, k, :],
                                                 h16[:, k * 5:(k + 1) * 5],
                                                 start=False, stop=False)
                            nc.tensor.matmul(dst, brz[0:1, m * 128:(m + 1) * 128],
                                             sb["ones15"][0:1, :], start=False, stop=True)
                        else:
                            nc.tensor.matmul(dst, bin_[0:1, (m - 8) * 128:(m - 7) * 128],
                                             sb["ones15"][0:1, :], start=False, stop=True)
                            dstn = phn[:, (m - 8) * 5:(m - 7) * 5]
                            for k in range(4):
                                nc.tensor.matmul(dstn, wh[:, m, k, :],
                                                 h16[:, k * 5:(k + 1) * 5],
                                                 start=(k == 0), stop=False)
                            nc.tensor.matmul(dstn, bhn[0:1, (m - 8) * 128:(m - 7) * 128],
                                             sb["ones15"][0:1, :], start=False, stop=True)
                    srz = wk.tile([128, 40], F32, tag="dsrz")
                    nc.scalar.activation(srz[:], prz[:], AF.Sigmoid)
                    tn = wk.tile([128, 20], F32, tag="dtn")
                    nc.vector.tensor_mul(tn[:], srz[:, 0:20], phn[:])
                    nc.vector.tensor_add(tn[:], tn[:], pin[:])
                    n_t = wk.tile([128, 20], F32, tag="dn")
                    nc.scalar.activation(n_t[:], tn[:], AF.Tanh)
                    d_t = wk.tile([128, 20], F32, tag="dd")
                    nc.vector.tensor_sub(d_t[:], hsl, n_t[:])
                    nc.vector.tensor_mul(d_t[:], srz[:, 20:40], d_t[:])
                    nc.vector.tensor_add(hsl, n_t[:], d_t[:])
                    nh16 = wk.tile([128, 20], F16, tag="nh16")
                    nc.vector.tensor_copy(nh16[:], hsl)
                    lin16 = nh16
                rnn16 = lin16

                # ---- attention ----
                pe_ = ps.tile([16, 128], F32, tag="pmid")
                r4 = rnn16.rearrange("p (k b) -> p k b", k=4)
                e4 = encOT16.rearrange("p (k t) -> p k t", k=4)
                for k in range(4):
                    nc.tensor.matmul(pe_[0:5, 0:T], r4[:, k, :], e4[:, k, :],
                                     start=(k == 0), stop=(k == 3))
                mx = wk.tile([16, 1], F32, tag="amx")
                nc.vector.tensor_reduce(mx[0:5, :], pe_[0:5, 0:T],
                                        axis=mybir.AxisListType.X, op=ALU.max,
                                        negate=True)
                aexp = wk.tile([16, 128], F32, tag="aexp")
                asum = wk.tile([16, 1], F32, tag="asum")
                nc.scalar.activation(aexp[0:5, 0:T], pe_[0:5, 0:T], AF.Exp,
                                     bias=mx[0:5, :], accum_out=asum[0:5, :])
                ars = wk.tile([16, 1], F32, tag="ars")
                nc.vector.reciprocal(ars[0:5, :], asum[0:5, :])
                nc.vector.tensor_scalar_mul(aexp[0:5, 0:T], aexp[0:5, 0:T],
                                            ars[0:5, :])
                pat = ps.tile([128, 8], F32, tag="pmid")
                nc.tensor.transpose(pat[:T, 0:5], aexp[0:5, 0:T], ident[0:5, 0:5])
                aT16 = wk.tile([128, 5], F16, tag="aT16")
                nc.vector.tensor_copy(aT16[0:T, :], pat[0:T, 0:5])
                pctx = ps.tile([128, 20], F32, tag="prz")
                for m in range(4):
                    nc.tensor.matmul(pctx[:, m * 5:(m + 1) * 5],
                                     encNat[:, m * 128:(m + 1) * 128],
                                     aT16[0:T, :], start=True, stop=True)
                ctx16 = wk.tile([128, 20], F16, tag="ctx16")
                nc.vector.tensor_copy(ctx16[:], pctx[:])

                # ---- Wc + tanh ----
                pcc = ps.tile([128, 20], F32, tag="pin")
                for m in range(4):
                    dst = pcc[:, m * 5:(m + 1) * 5]
                    for k in range(8):
                        rhs = rnn16[:, (k % 4) * 5:(k % 4) * 5 + 5] if k < 4 else \
                            ctx16[:, (k - 4) * 5:(k - 3) * 5]
                        nc.tensor.matmul(dst, Wc4[:, m, k, :], rhs,
                                         start=(k == 0), stop=False)
                    nc.tensor.matmul(dst, sb["bcr"][0:1, m * 128:(m + 1) * 128],
                                     sb["ones15"][0:1, :], start=False, stop=True)
                ccT = wk.tile([128, 20], F32, tag="ccT")
                nc.scalar.activation(ccT[:], pcc[:], AF.Tanh)

                # ---- Wout (fp32, vocab shard) ----
                pA = ps.tile([128, 500], F32, tag="pA")
                pB = ps.tile([128, 500], F32, tag="pB")
                for k in range(4):
                    for c in range(8):
                        pt_ = pA if c < 4 else pB
                        dst = pt_[32 * (c % 4):32 * (c % 4) + 5, :]
                        nc.tensor.matmul(dst, ccT[:, k * 5:(k + 1) * 5],
                                         WoutR[:, k, c * 500:(c + 1) * 500],
                                         start=(k == 0), stop=(k == 3),
                                         tile_position=(0, 32 * (c % 4)))
                nc.vector.tensor_add(pA[:], pA[:], sb["boutRep"][:, 0:500])
                nc.vector.tensor_add(pB[:], pB[:], sb["boutRep"][:, 500:1000])

                eA = wk.tile([128, 500], F32, tag="eA")
                sA = wk.tile([128, 1], F32, tag="sA")
                nc.scalar.activation(eA[:], pA[:], AF.Exp, accum_out=sA[:])
                eB = wk.tile([128, 500], F32, tag="eB")
                sB = wk.tile([128, 1], F32, tag="sB")
                nc.scalar.activation(eB[:], pB[:], AF.Exp, accum_out=sB[:])
                rA = wk.tile([128, 500], F32, tag="rA")
                nc.vector.tensor_copy(rA[:], pA[:])
                rB = wk.tile([128, 500], F32, tag="rB")
                nc.vector.tensor_copy(rB[:], pB[:])

                v8A = wk.tile([128, 8], F32, tag="v8A")
                i8A = wk.tile([128, 8], U32, tag="i8A")
                nc.vector.max(v8A[:], rA[:])
                nc.vector.max_index(i8A[:], v8A[:], rA[:])
                v8B = wk.tile([128, 8], F32, tag="v8B")
                i8B = wk.tile([128, 8], U32, tag="i8B")
                nc.vector.max(v8B[:], rB[:])
                nc.vector.max_index(i8B[:], v8B[:], rB[:])
                gfA = wk.tile([128, 8], F32, tag="gfA")
                nc.vector.tensor_copy(gfA[:], i8A[:])
                nc.vector.tensor_scalar_add(gfA[:], gfA[:], sb["voffA"][:])
                gfB = wk.tile([128, 8], F32, tag="gfB")
                nc.vector.tensor_copy(gfB[:], i8B[:])
                nc.vector.tensor_scalar_add(gfB[:], gfB[:], sb["voffB"][:])

                # pre-AG merge -> [5, 129]
                pm = ps.tile([16, 136], F32, tag="pmid")
                for c in range(4):
                    nc.tensor.matmul(pm[0:5, c * 8:(c + 1) * 8],
                                     sb["Icc"][:, c * 5:(c + 1) * 5], v8A[:],
                                     start=True, stop=True)
                    nc.tensor.matmul(pm[0:5, 32 + c * 8:32 + (c + 1) * 8],
                                     sb["Icc"][:, c * 5:(c + 1) * 5], v8B[:],
                                     start=True, stop=True)
                    nc.tensor.matmul(pm[0:5, 64 + c * 8:64 + (c + 1) * 8],
                                     sb["Icc"][:, c * 5:(c + 1) * 5], gfA[:],
                                     start=True, stop=True)
                    nc.tensor.matmul(pm[0:5, 96 + c * 8:96 + (c + 1) * 8],
                                     sb["Icc"][:, c * 5:(c + 1) * 5], gfB[:],
                                     start=True, stop=True)
                nc.tensor.matmul(pm[0:5, 128:129], sb["Isum"][:], sA[:],
                                 start=True, stop=False)
                nc.tensor.matmul(pm[0:5, 128:129], sb["Isum"][:], sB[:],
                                 start=False, stop=True)
                nc.vector.tensor_copy(payload[:, 0:129], pm[0:5, 0:129])

                # AllGather
                NOCC = bool(os.environ.get("K_NOCC"))
                agi = dramp.tile([5, 136], F32, tag="agi")
                ago = dramp.tile([NCORE * 5, 136], F32, tag="ago",
                                 addr_space="Shared")
                nc.gpsimd.dma_start(agi[:], payload[:])
                if NOCC:
                    nc.gpsimd.dma_start(ago[0:5, :], agi[:])
                else:
                    nc.gpsimd.collective_compute(
                        "AllGather", ALU.bypass,
                        ins=[agi.opt()], outs=[ago.opt()],
                        replica_groups=[list(range(NCORE))])
                nc.gpsimd.dma_start(gsb[:], ago[:])

                # post-AG merge: per-beam 512 candidates
                pqv = ps.tile([16, 512], F32, tag="pA")
                pqg = ps.tile([16, 512], F32, tag="pB")
                pqs = ps.tile([16, 1], F32, tag="pqs")
                for r in range(NCORE):
                    nc.tensor.matmul(pqv[0:5, r * 64:(r + 1) * 64],
                                     sb["Ir"][:, r * 5:(r + 1) * 5],
                                     gsb[:, 0:64], start=True, stop=True)
                    nc.tensor.matmul(pqg[0:5, r * 64:(r + 1) * 64],
                                     sb["Ir"][:, r * 5:(r + 1) * 5],
                                     gsb[:, 64:128], start=True, stop=True)
                nc.tensor.matmul(pqs[0:5, :], sb["IrSum"][:], gsb[:, 128:129],
                                 start=True, stop=True)
                nc.vector.tensor_copy(mv[0:5, 0:512], pqv[0:5, :])
                nc.vector.tensor_copy(mv[0:5, 512:1024], pqg[0:5, :])
                nc.scalar.activation(lse[0:5, :], pqs[0:5, :], AF.Ln)

                cfull = wk.tile([16, 512], F32, tag="cfull")
                nc.vector.tensor_scalar_sub(cfull[:], mv[:, 0:512], lse[:])
                nc.vector.tensor_scalar_add(cfull[:], cfull[:], scores[:])
                for b in range(5):
                    pfl = ps.tile([16, 512], F32, tag="pqs")
                    nc.tensor.matmul(pfl[0:1, :], sb["Ib"][:, b:b + 1],
                                     cfull[:], start=True, stop=True)
                    nc.vector.tensor_copy(cflat[0:1, b * 512:(b + 1) * 512],
                                          pfl[0:1, :])
                    pfl2 = ps.tile([16, 512], F32, tag="pfg")
                    nc.tensor.matmul(pfl2[0:1, :], sb["Ib"][:, b:b + 1],
                                     mv[:, 512:1024], start=True, stop=True)
                    nc.vector.tensor_copy(gflat[0:1, b * 512:(b + 1) * 512],
                                          pfl2[0:1, :])
                nc.sync.dma_start(out=gdram[:], in_=gflat[:])

                ft = wk.tile([1, 8], F32, tag="ft")
                nc.vector.max(ft[:], cflat[:])
                fp_ = wk.tile([1, 8], U32, tag="fp_")
                nc.vector.max_index(fp_[:], ft[:], cflat[:])
                parw = wk.tile([1, 8], U32, tag="parw")
                nc.vector.tensor_scalar(parw[:], fp_[:], 9, None,
                                        ALU.logical_shift_right)
                parf = wk.tile([1, 8], F32, tag="parf")
                nc.vector.tensor_copy(parf[:], parw[:])

                # scores + parents to columns via transpose
                nc.vector.tensor_copy(tt10[0:1, 0:5], ft[0:1, 0:5])
                nc.vector.tensor_copy(tt10[0:1, 5:10], parf[0:1, 0:5])
                ptr = ps.tile([16, 8], F32, tag="pqs")
                nc.tensor.transpose(ptr[0:5, 0:1], tt10[0:1, 0:5], ident[0:1, 0:1])
                nc.tensor.transpose(ptr[0:5, 1:2], tt10[0:1, 5:10], ident[0:1, 0:1])
                nc.vector.tensor_copy(scores[0:5, :], ptr[0:5, 0:1])
                nc.vector.tensor_copy(parsAcc[:, st:st + 1], ptr[:, 1:2])
                # winner positions -> column, then vocab ids via indirect DMA
                pposf = wk.tile([1, 8], F32, tag="pposf")
                nc.vector.tensor_copy(pposf[:], fp_[:])
                ptr2 = ps.tile([16, 8], F32, tag="pfg")
                nc.tensor.transpose(ptr2[0:5, 0:1], pposf[0:1, 0:5],
                                    ident[0:1, 0:1])
                posc = wk.tile([16, 1], I32, tag="posc")
                nc.gpsimd.memset(posc[:], 0)
                nc.vector.tensor_copy(posc[0:5, :], ptr2[0:5, 0:1])
                tokc = wk.tile([16, 1], F32, tag="tokc")
                nc.gpsimd.indirect_dma_start(
                    out=tokc[:], out_offset=None, in_=gdram[:],
                    in_offset=bass.IndirectOffsetOnAxis(ap=posc[:, 0:1], axis=0),
                    bounds_check=5 * 512 - 1, oob_is_err=False)
                nc.vector.tensor_copy(toksAcc[:, st:st + 1], tokc[:])
                idxf = wk.tile([16, 1], F32, tag="idxf")
                nc.vector.tensor_copy(idxf[:], tokc[:])
                nc.vector.tensor_copy(idxt[0:5, :], idxf[0:5, :])

                if st == S - 1:
                    break

                # embedding gather for new tokens
                pemb = wk.tile([16, 512], F16, tag="pemb")
                nc.gpsimd.indirect_dma_start(
                    out=pemb[:], out_offset=None, in_=din["emb"][:],
                    in_offset=bass.IndirectOffsetOnAxis(ap=idxt[:, 0:1], axis=0),
                    bounds_check=V - 1, oob_is_err=False)
                pxt = ps.tile([128, 32], F16, tag="pmid")
                for k in range(4):
                    nc.tensor.transpose(pxt[:, k * 8:k * 8 + 5],
                                        pemb[0:5, k * 128:(k + 1) * 128],
                                        sb["ident16"][0:5, 0:5])
                nc.vector.tensor_copy(
                    xdT[:].rearrange("p (k b) -> p k b", k=4),
                    pxt[:].rearrange("p (k c) -> p k c", k=4)[:, :, 0:5])

                # hidden reorder by parent (transpose -> permute-matmul -> transpose)
                pth = ps.tile([64, 128], F32, tag="prz")
                nc.tensor.transpose(pth[0:40, :], hdT[:], ident[:])
                th = wk.tile([64, 128], F32, tag="th")
                nc.vector.tensor_copy(th[0:40, :], pth[0:40, :])
                tgt = wk.tile([1, 40], F32, tag="tgt")
                parv = parf[0:1, 0:5].unsqueeze(1).broadcast_to([1, 8, 5])
                nc.vector.tensor_tensor(tgt[0:1, :].rearrange("a (c b) -> a c b", b=5),
                                        sb["base40"][0:1, :].rearrange("a (c b) -> a c b", b=5),
                                        parv, op=ALU.add)
                ptgt = ps.tile([64, 40], F32, tag="pin")
                nc.tensor.matmul(ptgt[0:40, :], sb["ones40"][0:1, 0:40],
                                 tgt[0:1, :], start=True, stop=True)
                P40 = wk.tile([64, 40], F32, tag="P40")
                nc.vector.tensor_tensor(
                    P40[0:40, :],
                    sb["rowidx40"][0:40, 0:1].broadcast_to([40, 40]),
                    ptgt[0:40, :],
                    op=ALU.is_equal)
                pnh = ps.tile([64, 128], F32, tag="prz")
                nc.tensor.matmul(pnh[0:40, :], P40[0:40, :], th[0:40, :],
                                 start=True, stop=True)
                th2 = wk.tile([64, 128], F32, tag="th2")
                nc.vector.tensor_copy(th2[0:40, :], pnh[0:40, :])
                ptb = ps.tile([128, 40], F32, tag="pmid")
                nc.tensor.transpose(ptb[:], th2[0:40, :], ident[0:40, 0:40])
                nc.vector.tensor_copy(hdT[:], ptb[:])

            nc.sync.dma_start(
                out=out_tp[0:1, :].rearrange("a (s b) -> (a b) s", b=5),
                in_=toksAcc[0:5, :])
            nc.sync.dma_start(
                out=out_tp[1:2, :].rearrange("a (s b) -> (a b) s", b=5),
                in_=parsAcc[0:5, :])
            nc.sync.dma_start(out=out_sc[:], in_=scores[0:5, :])
    nc.compile()
    return nc


def _prep_inputs(inputs, T, S):
    f32 = np.float32
    emb = np.asarray(inputs["emb"], f32)
    seq = np.asarray(inputs["input_seq"]).reshape(-1).astype(np.int64)
    sos = int(np.asarray(inputs["SOS_token"]))
    g = {}
    for nm in ["enc_Wi", "enc_Wh", "enc_bi", "enc_bh",
               "dec_Wi", "dec_Wh", "dec_bi", "dec_bh"]:
        g[nm] = np.asarray(inputs[nm], f32)
    Wc = np.asarray(inputs["Wc"], f32)
    bc = np.asarray(inputs["bc"], f32)
    Wout = np.asarray(inputs["Wout"], f32)
    bout = np.asarray(inputs["bout"], f32)

    d = {}
    for lay in range(2):
        d[f"encWh{lay}"] = _tile_wT(g["enc_Wh"][lay]).astype(np.float16)
        d[f"encWi{lay}"] = _tile_wT(g["enc_Wi"][lay]).astype(np.float16)
        d[f"decWi{lay}"] = _tile_wT(g["dec_Wi"][lay]).astype(np.float16)
        d[f"decWh{lay}"] = _tile_wT(g["dec_Wh"][lay]).astype(np.float16)
    for lay in range(2):
        bi, bh = g["enc_bi"][lay], g["enc_bh"][lay]
        bb = bi.copy()
        bb[:1024] += bh[:1024]
        d[f"enc_gib{lay}"] = bb.reshape(1, -1)
        d[f"enc_bhn{lay}"] = bh[1024:].reshape(4, 128).T.copy()
        dbi, dbh = g["dec_bi"][lay], g["dec_bh"][lay]
        d[f"dec_brz{lay}"] = (dbi[:1024] + dbh[:1024]).reshape(1, -1)
        d[f"dec_bin{lay}"] = dbi[1024:].reshape(1, -1)
        d[f"dec_bhn{lay}"] = dbh[1024:].reshape(1, -1)
    d["WcT"] = _tile_wT(Wc).astype(np.float16)
    d["bcr"] = bc.reshape(1, -1)
    d["ident"] = np.eye(128, dtype=f32)
    d["ident16"] = np.eye(128, dtype=np.float16)
    d["ones15"] = np.ones((1, 5), f32)
    d["onesT"] = np.ones((1, T), f32)
    x = emb[seq]  # [T, H]
    d["xT"] = np.ascontiguousarray(
        x.T.reshape(4, 128, T).transpose(1, 0, 2).reshape(128, 4 * T)
    ).astype(np.float16)
    xd = np.broadcast_to(emb[sos], (5, 512)).T  # [512,5]
    d["xd0"] = np.ascontiguousarray(
        xd.reshape(4, 128, 5).transpose(1, 0, 2).reshape(128, 20)).astype(np.float16)
    Icc = np.zeros((128, 20), f32)
    for c in range(4):
        for b in range(5):
            Icc[32 * c + b, c * 5 + b] = 1.0
    d["Icc"] = Icc
    d["Isum"] = Icc.reshape(128, 4, 5).sum(1)
    Ir = np.zeros((40, 40), f32)
    for r in range(8):
        for b in range(5):
            Ir[5 * r + b, r * 5 + b] = 1.0
    d["Ir"] = Ir
    d["IrSum"] = Ir.reshape(40, 8, 5).sum(1)
    Ib = np.zeros((16, 5), f32)
    for b in range(5):
        Ib[b, b] = 1.0
    d["Ib"] = Ib
    d["base40"] = (np.arange(40) // 5 * 5).astype(f32).reshape(1, 40)
    d["rowidx40"] = np.arange(64, dtype=f32).reshape(64, 1)
    d["ones40"] = np.ones((1, 40), f32)
    h0 = np.zeros((16, 1), f32)
    h0[1:5] = NEG
    d["h0sc"] = h0
    d["emb"] = emb.astype(np.float16)

    # per-core tensors
    per_core = []
    WoutT = Wout.T  # [H, V]
    for r in range(NCORE):
        dc = dict(d)
        sh = WoutT[:, r * VS:(r + 1) * VS]  # [512, 4000]
        dc["WoutT"] = np.ascontiguousarray(
            sh.reshape(4, 128, VS).transpose(1, 0, 2).reshape(128, 4 * VS))
        bsh = bout[r * VS:(r + 1) * VS]
        br = np.zeros((128, 1000), f32)
        for c in range(8):
            half = c // 4
            for b in range(5):
                br[32 * (c % 4) + b, half * 500:(half + 1) * 500] = bsh[c * 500:(c + 1) * 500]
        dc["boutRep"] = br
        voffA = np.zeros((128, 1), f32)
        voffB = np.zeros((128, 1), f32)
        for c in range(4):
            voffA[32 * c:32 * (c + 1)] = r * VS + c * 500
            voffB[32 * c:32 * (c + 1)] = r * VS + (c + 4) * 500
        dc["voffA"] = voffA
        dc["voffB"] = voffB
        per_core.append(dc)
    return per_core


def kernel(**inputs):
    T = int(np.asarray(inputs["input_length"]))
    S = int(np.asarray(inputs["max_length"]))
    key = (T, S)
    if key not in _CACHE:
        _CACHE[key] = build(T, S)
    nc = _CACHE[key]
    in_maps = _prep_inputs(inputs, T, S)
    res = run_bass_kernel_spmd(nc, in_maps, list(range(NCORE)))
    r0 = res.results[0]
    toks = r0["out_tp"][0].reshape(S, 5).astype(np.int64)
    pars = r0["out_tp"][1].reshape(S, 5).astype(np.int64)
    scores = r0["out_sc"].reshape(5)
    best = int(np.argmax(scores))
    seq = np.zeros(S, np.int64)
    b = best
    for t in range(S - 1, -1, -1):
        seq[t] = toks[t, b]
        b = pars[t, b]
    in_seq = np.asarray(inputs["input_seq"])
    seq = seq.astype(in_seq.dtype) if np.issubdtype(in_seq.dtype, np.integer) else seq.astype(np.int32)
    return seq, np.float32(scores[best])
